# revision 36
# baseline (speedup 1.0000x reference)
"""Multi-head attention (B=2, S=2048, D=1024, H=16, causal) on 8 TRN2 NeuronCores.

Sharding: core c handles batch c//4 and heads [4*(c%4), 4*(c%4)+4) —
data-parallel over batch x tensor-parallel over heads, Megatron-style:
QKV projection weights are column-split (each core computes only its own
heads' features), the output projection is row-split (each core emits a
full-width partial that the host sums).

Per-core device kernel (bf16 matmul operands, fp32 accumulation):
  - Q,K projected feature-major (QT/KT = W_local @ x^T, shape (256, 2048))
    so the scores matmul needs no on-device transposes.
  - V projected in natural (seq, feat) layout with a fused ones-column so
    a single PV matmul produces both attn@V and the softmax denominator.
  - scores^T per (head, q-chunk, key-chunk): K^T-chunk stationary, Q moving.
  - softmax without max-subtraction (scores ~ N(0,1); exp is accurate
    enough), causal handled by skipping upper-triangle key chunks and
    affine_select-masking the 4 diagonal chunk patterns.
  - normalization: the PV accumulator is copied to SBUF immediately (frees
    the PSUM slot so the PE never stalls), all 1/Z reciprocals of a chunk
    run on VectorE at the chunk boundary (DVE reciprocal — ScalarE Ln/Exp
    thrash ACT table sets; custom-DVE ops and partition_broadcast
    mis-execute on HW via this path), and Z is broadcast across 64
    partitions with a K=64 one-hot fp32 matmul (K=1 matmuls read as idle
    to the PE activity monitor and re-throttled the clock).
  - O projection contracts the 256 local features against Wo rows; the
    partial output is written feature-major (1024, 2048) fp32 and the
    host transposes/sums partials and adds bo.

Scheduling notes (measured on HW via neuron-profile): input rows stream as
full 512KB DMAs (4KB/partition descriptors), weights ship pre-arranged for
contiguous DMA, dependency-less warm-up matmuls run during the initial DMAs
so the HAM clock gate is at 8/8 when real work starts, and the K/V/O
projections are interleaved chunk-wise with the attention chunks (causality
only needs K columns and V chunks progressively) as PE-dense filler where
the exp(ACT)-paced attention pipeline would otherwise idle the PE.
"""

import numpy as np
import ml_dtypes

import concourse.bacc as bacc
import concourse.mybir as mybir
import concourse.tile as tile
from concourse.bass_utils import run_bass_kernel_spmd

B, S, D, H = 2, 2048, 1024, 16
DK = D // H           # 64, head dim
DL = 256              # local (per-core) projected features = 4 heads
NHL = 4               # heads per core
NQ = 4                # q-chunks of 512
F32 = mybir.dt.float32
BF16 = mybir.dt.bfloat16
NPBF16 = ml_dtypes.bfloat16


def _emit(tc, io):
    nc = tc.nc
    qt, kt, vt = io["qt"], io["kt"], io["vt"]          # (1024, 2048) bf16
    wqt, wkt, wvt = io["wqt"], io["wkt"], io["wvt"]    # (1024, 256) bf16
    wot = io["wot"]                                    # (256, 1024) bf16
    bqc, bkc = io["bqc"], io["bkc"]                    # (128, 2) f32
    bvr = io["bvr"]                                    # (1, 256) bf16
    outp = io["outp"]                                  # (1024, 2048) bf16 partials
    EXP = mybir.ActivationFunctionType.Exp

    with (
        tc.tile_pool(name="const", bufs=1) as cw,
        tc.tile_pool(name="io", bufs=16) as iop,
        tc.tile_pool(name="big", bufs=1) as big,
        tc.tile_pool(name="work", bufs=3) as wk,
        tc.tile_pool(name="psA", bufs=2, space="PSUM") as psA,
        tc.tile_pool(name="psB", bufs=2, space="PSUM") as psB,
        tc.tile_pool(name="psC", bufs=2, space="PSUM") as psC,
    ):
        ones_sb = cw.tile([128, 128], BF16)
        nc.vector.memset(ones_sb[:], 1.0)
        bq_sb = cw.tile([128, 2], F32)
        nc.sync.dma_start(bq_sb[:], bqc[:, :])
        bk_sb = cw.tile([128, 2], F32)
        nc.sync.dma_start(bk_sb[:], bkc[:, :])
        bv_sb = cw.tile([1, 256], BF16)
        nc.sync.dma_start(bv_sb[:], bvr[:, :])

        # only the Q weights up front — the other weight DMAs are emitted
        # right before their phase so the first projection matmuls start ASAP
        wq_sb = cw.tile([128, 8, 256], BF16)
        nc.sync.dma_start(wq_sb[:], wqt[:, :].rearrange("p (k m) -> p k m", m=256))

        # free PE warm-up: dependency-less matmuls run while the first
        # weight/row DMAs are in flight, so the HAM clock gate is already at
        # 8/8 when the real work begins
        warm = cw.tile([128, 512], BF16, name="warm")
        nc.vector.memset(warm[:], 0.0)
        for _ in range(16):
            wps = psC.tile([128, 512], F32, tag="pv", name="wps")
            nc.tensor.matmul(wps[:], ones_sb[:, :], warm[:], start=True, stop=True)

        QT = big.tile([128, 2, S], BF16)   # [feat%128, feat//128, seq]
        # K^T kept as two half-zeroed copies so the scores matmul contracts
        # over the full 128 partitions (zeros kill the other head's Q rows);
        # K=64 matmuls read as "half-idle" to the PE activity monitor and the
        # clock gate kept re-throttling the whole attention phase.
        KTe = big.tile([128, 2, S], BF16)
        KTo = big.tile([128, 2, S], BF16)
        nc.gpsimd.memset(KTe[64:128, :, :], 0.0)
        nc.gpsimd.memset(KTo[0:64, :, :], 0.0)
        # [key%128, head, key//128, dk | ones | zero-pad]: padded to 128 cols
        # so PV matmuls drive the full array (M=65 looked half-idle to the PE
        # activity monitor); col 64 is the softmax-denominator ones column
        VA = big.tile([128, NHL, 16, 128], BF16)
        ON = big.tile([128, 2, S], BF16)   # normalized attn out, feature-major
        nc.gpsimd.memset(VA[:, :, :, 64:128], 0.0)
        nc.gpsimd.memset(VA[:, :, :, 64:65], 1.0)

        # Z-reciprocal batching: heads' Z rows are gathered onto distinct
        # partitions of one tile so a single DVE RECIPROCAL (3.3us, free-size
        # bound) serves 4 heads instead of one 3.3us op per head. Gathered
        # partitions sit inside an all-ones tile so the K=64 broadcast matmul
        # contracts 1.0*0.0 (not inf*0 = NaN) on the unused partitions.
        # (single-partition writes must sit at 32-aligned partition offsets,
        # so the gathered Z rows live at partitions 32i)
        zb4 = cw.tile([128, 512], F32)  # batched Z rows at partition 32i
        nc.gpsimd.memset(zb4[:], 1.0)
        zbB = cw.tile([64, 512], F32)   # j=3 solo head 3: Z at partition 0
        nc.gpsimd.memset(zbB[:], 1.0)
        # one-hots + 1/Z operands for the broadcast matmul are BF16: an fp32
        # matmul lowers to TWO half-speed LOW/HIGH passes (~2.1us apiece on
        # the PE) vs 213ns for the bf16 one; 1/Z at bf16 costs ~0.2% rms
        e4 = cw.tile([128, 256], BF16)  # one-hot blocks: col block h row 32h
        nc.gpsimd.memset(e4[:], 0.0)
        for h in range(NHL):
            nc.gpsimd.memset(e4[32 * h : 32 * h + 1, 64 * h : 64 * h + 64], 1.0)
        e2 = cw.tile([64, 128], BF16)   # one-hot blocks: col block i row 32i
        nc.gpsimd.memset(e2[:], 0.0)
        for i in range(2):
            nc.gpsimd.memset(e2[32 * i : 32 * i + 1, 64 * i : 64 * i + 64], 1.0)
        zb4b = cw.tile([128, 512], BF16)  # bf16 casts of the 1/Z tiles
        zbBb = cw.tile([64, 512], BF16)
        zcast = {id(zb4): zb4b, id(zbB): zbBb}

        # ---- Q/K projections, feature-major: dst[:, m, n] = W_local @ x^T ----
        # inputs stream as full 512KB rows (4KB/partition descriptors — small
        # per-partition runs were tanking HW-DGE efficiency)
        wk_sb = cw.tile([128, 8, 256], BF16)
        wv_sb = cw.tile([128, 8, 256], BF16)
        wo_sb = cw.tile([128, 2, 1024], BF16)
        # inputs live in one [128, 8, S] tile each so a single multi-dim DMA
        # moves a whole column chunk of all 8 k-rows (1 issue instead of 8 —
        # SP issue cost is ~0.6us per descriptor). q streams in quarters on
        # the (early-idle) Scalar DMA queue: Qproj chunk n is gated only on
        # quarter n, so the PE gets real work ~4.5us in instead of ~15us.
        # per-(row, half) DMAs: a fused [128,8,512] DMA looked elegant but
        # its 1024-descriptor issue takes 3.6-5us on the queue (vs 0.6us for
        # a contiguous per-row slice), which starved the first projections
        qrs = iop.tile([128, 8, S], BF16, tag="xin", name="qrs", bufs=2)
        for hf in range(2):
            hsl = slice(hf * 1024, (hf + 1) * 1024)
            for k in range(8):
                nc.scalar.dma_start(
                    qrs[:, k, hsl], qt[k * 128 : (k + 1) * 128, hsl]
                )
        for n in range(NQ):
            pm = [
                psA.tile([128, 512], F32, tag="proj", name=f"pm{m}")
                for m in range(2)
            ]
            for k in range(8):
                for m in range(2):
                    nc.tensor.matmul(
                        pm[m][:],
                        wq_sb[:, k, m * 128 : (m + 1) * 128],
                        qrs[:, k, n * 512 : (n + 1) * 512],
                        start=(k == 0),
                        stop=(k == 7),
                    )
            for m in range(2):
                nc.vector.tensor_scalar_add(
                    QT[:, m, n * 512 : (n + 1) * 512], pm[m][:], bq_sb[:, m : m + 1]
                )

        # K projection split per q-chunk: attention chunk j only needs K
        # columns up to (j+1)*512, so chunks n>=1 are emitted between the
        # attention chunks below (PE-dense filler for the exp-paced phase)
        nc.sync.dma_start(wk_sb[:], wkt[:, :].rearrange("p (k m) -> p k m", m=256))
        krs = iop.tile([128, 8, S], BF16, tag="xin", name="krs", bufs=2)
        for hf in range(2):
            hsl = slice(hf * 1024, (hf + 1) * 1024)
            for k in range(8):
                nc.sync.dma_start(
                    krs[:, k, hsl], kt[k * 128 : (k + 1) * 128, hsl]
                )

        def emit_kproj(n):
            pm = [
                psA.tile([128, 512], F32, tag="proj", name=f"km{m}")
                for m in range(2)
            ]
            for k in range(8):
                for m in range(2):
                    nc.tensor.matmul(
                        pm[m][:],
                        wk_sb[:, k, m * 128 : (m + 1) * 128],
                        krs[:, k, n * 512 : (n + 1) * 512],
                        start=(k == 0),
                        stop=(k == 7),
                    )
            for m in range(2):
                sl = slice(n * 512, (n + 1) * 512)
                nc.vector.tensor_scalar_add(
                    KTe[0:64, m, sl], pm[m][0:64, :], bk_sb[0:64, m : m + 1]
                )
                nc.vector.tensor_scalar_add(
                    KTo[64:128, m, sl], pm[m][64:128, :], bk_sb[64:128, m : m + 1]
                )

        emit_kproj(0)

        # ---- V projection, natural layout, bias via K=1 ones matmul ----
        # emitted in sp-pairs interleaved with the attention chunks below:
        # attention is exp(ACT)-paced, so V matmuls fill the PE micro-idles
        # that would otherwise re-throttle the clock gate
        nc.sync.dma_start(wv_sb[:], wvt[:, :].rearrange("p (k m) -> p k m", m=256))
        # vrs shares the xin ring with qrs: the v DMAs wait out Qproj's last
        # read (~15us) and still land well before vproj needs them (~25us).
        # Quarters: vproj sp-pair (2n, 2n+1) needs only v columns quarter n.
        vrs = iop.tile([128, 8, S], BF16, tag="xin", name="vrs", bufs=2)
        for hf in range(2):
            hsl = slice(hf * 1024, (hf + 1) * 1024)
            for k in range(8):
                nc.sync.dma_start(
                    vrs[:, k, hsl], vt[k * 128 : (k + 1) * 128, hsl]
                )

        def emit_vproj(sps):
            for sp in sps:
                pvps = psA.tile([128, 512], F32, tag="proj", name="pvps")
                for k in range(8):
                    for half in range(2):
                        s = sp * 256 + half * 128
                        nc.tensor.matmul(
                            pvps[:, half * 256 : (half + 1) * 256],
                            vrs[:, k, s : s + 128],
                            wv_sb[:, k, :],
                            start=(k == 0 and half == 0),
                            stop=False,
                        )
                for half in range(2):
                    nc.tensor.matmul(
                        pvps[:, half * 256 : (half + 1) * 256],
                        ones_sb[0:1, 0:128],
                        bv_sb[:],
                        start=False,
                        stop=(half == 1),
                    )
                for half in range(2):
                    s = sp * 2 + half
                    nc.vector.tensor_copy(
                        VA[:, :, s, 0:64],
                        pvps[:, half * 256 : (half + 1) * 256].rearrange(
                            "p (h d) -> p h d", d=64
                        ),
                    )

        emit_vproj((0, 1))

        # wo is not needed until oproj(0) (~55us): defer its DMA out of
        # the congested first-25us HBM window
        nc.sync.dma_start(wo_sb[:], wot[:, :].rearrange("p (c m) -> p c m", m=1024))

        # ---- attention + output projection, q-chunk-major for overlap ----
        # O-proj for chunk j is emitted after attention chunk j+1 so the PE
        # never waits on the normalize chain (reciprocal on DVE) of chunk j.
        def emit_oproj(j, ms=range(8), on_act=False):
            for m in ms:
                po = psA.tile([128, 512], F32, tag="proj", name="po")
                for c in range(2):
                    nc.tensor.matmul(
                        po[:],
                        wo_sb[:, c, m * 128 : (m + 1) * 128],
                        ON[:, c, j * 512 : (j + 1) * 512],
                        start=(c == 0),
                        stop=(c == 1),
                    )
                # bf16 partials: host upcasts before the 4-way sum; halves
                # the 8MB/core output DMA and the end-of-kernel drain
                ot = wk.tile([128, 512], BF16, tag="ot", name="ot")
                if on_act:
                    # the last two O-projections run after the exps are done:
                    # their PSUM-freeing copies go on the then-idle ScalarE
                    # (COPY is in every ACT table set - no thrash) so the PE
                    # never paces on the DVE queue at the tail
                    nc.scalar.activation(
                        ot[:], po[:], mybir.ActivationFunctionType.Copy
                    )
                    nc.gpsimd.dma_start(
                        outp[m * 128 : (m + 1) * 128, j * 512 : (j + 1) * 512],
                        ot[:],
                    )
                else:
                    nc.vector.tensor_copy(ot[:], po[:])
                    nc.sync.dma_start(
                        outp[m * 128 : (m + 1) * 128, j * 512 : (j + 1) * 512],
                        ot[:],
                    )

        # normalize a batch of heads, split in two so the PE-queue placement
        # of the broadcast matmuls can be decoupled from the reciprocal:
        #   pre:  gather Z rows onto distinct partitions of zb (tiny
        #         SBUF->SBUF DMAs — engines can't cross partitions) + one
        #         batched DVE reciprocal (3.3us, free-size bound) for the
        #         whole batch instead of one 3.3us reciprocal per head
        #   post: K=64/128 one-hot matmul broadcast + DVE scale into ON
        def emit_norm_pre(us, heads, zb, stride):
            for i, h in enumerate(heads):
                nc.sync.dma_start(
                    zb[stride * i : stride * i + 1, :], us[h][64:65, :]
                )
            # reciprocal converts to bf16 on the way out (DVE auto-converts)
            # so the broadcast matmul gets its full-speed operand directly
            with nc.allow_low_precision(reason="1/Z bf16 broadcast operand"):
                nc.vector.reciprocal(zcast[id(zb)][:], zb[:])

        def emit_norm_post(j, us, heads, zb, eh):
            for i, h in enumerate(heads):
                rbps = psA.tile([64, 512], F32, tag="proj", name="rbps")
                nc.tensor.matmul(
                    rbps[:],
                    eh[:, 64 * i : 64 * i + 64],
                    zcast[id(zb)][:],
                    start=True,
                    stop=True,
                )
                if h % 2 == 0:
                    nc.vector.tensor_mul(
                        ON[0:64, h // 2, j * 512 : (j + 1) * 512], us[h][0:64, :], rbps[:]
                    )
                else:
                    nt = wk.tile([64, 512], BF16, tag="nt", name="nt")
                    nc.vector.tensor_mul(nt[:], us[h][0:64, :], rbps[:])
                    nc.sync.dma_start(
                        ON[64:128, h // 2, j * 512 : (j + 1) * 512], nt[:]
                    )

        for j in range(NQ):
            nkc = 4 * (j + 1)
            us = []
            for h in range(NHL):
                ch = h // 2
                pvp = psC.tile([128, 512], F32, tag="pv", name="pvp")
                # exp batched over 2 key-chunks (one 2-bank PSUM tile) to
                # amortize the ~350ns fixed cost per ACTIVATE. Diagonal
                # key-chunks (kc >= 4j) skip their leading fully-masked
                # columns: q columns < 128*(kc-4j) attend only to earlier
                # keys, so scores/exp/select/PV all trim to [c0:512].
                for kp in range(nkc // 2):
                    scp = psB.tile([128, 1024], F32, tag="sc", name="scp")
                    ktz = KTe if h % 2 == 0 else KTo
                    c0s = []
                    for half in range(2):
                        kc = 2 * kp + half
                        c0 = max(0, 128 * (kc - 4 * j))
                        c0s.append(c0)
                        nc.tensor.matmul(
                            scp[:, half * 512 + c0 : (half + 1) * 512],
                            ktz[:, ch, kc * 128 : (kc + 1) * 128],
                            QT[:, ch, j * 512 + c0 : (j + 1) * 512],
                            start=True,
                            stop=True,
                        )
                    et = wk.tile([128, 1024], BF16, tag="exp", name="et", bufs=4)
                    if c0s[1] == 0:
                        nc.scalar.activation(
                            et[:], scp[:], EXP, scale=float(DK) ** -0.5
                        )
                    else:
                        for half in range(2):
                            sl = slice(half * 512 + c0s[half], (half + 1) * 512)
                            nc.scalar.activation(
                                et[:, sl], scp[:, sl], EXP, scale=float(DK) ** -0.5
                            )
                    for half in range(2):
                        kc = 2 * kp + half
                        if kc >= 4 * j:
                            c0 = c0s[half]
                            nc.gpsimd.affine_select(
                                out=et[:, half * 512 + c0 : (half + 1) * 512],
                                in_=et[:, half * 512 + c0 : (half + 1) * 512],
                                compare_op=mybir.AluOpType.is_ge,
                                fill=0.0,
                                base=0,
                                pattern=[[1, 512 - c0]],
                                channel_multiplier=-1,
                            )
                    for half in range(2):
                        kc = 2 * kp + half
                        c0 = c0s[half]
                        nc.tensor.matmul(
                            pvp[:, c0:512],
                            VA[:, h, kc, :],
                            et[:, half * 512 + c0 : (half + 1) * 512],
                            start=(kc == 0),
                            stop=(kc == nkc - 1),
                        )
                # copy the PV accumulator to SBUF immediately — freeing the
                # pv PSUM slot fast keeps the PE from stalling (and HAM from
                # re-throttling the clock); normalization is deferred below.
                u = wk.tile([128, 512], F32, tag="u", name="u", bufs=6)
                nc.vector.tensor_copy(u[:], pvp[:])
                us.append(u)
                # last chunk: normalize in 2-head batches, pre (reciprocal)
                # right after the pair's PVs land and post (PE broadcast)
                # only after another head's attention has filled the PE
                # queue, so the O-projections aren't serialized behind one
                # end-of-chunk reciprocal
                if j == NQ - 1 and h == 2:
                    # heads 0-2 batch: reciprocal runs during h3's attention
                    emit_norm_pre(us, (0, 1, 2), zb4, 32)
                if j == NQ - 1 and h == 3:
                    # solo head 3: column-split reciprocal so the first half
                    # of 1/Z is ready ~1.7us sooner on the tail chain
                    nc.sync.dma_start(zbB[0:1, :], us[3][64:65, :])
                    with nc.allow_low_precision(reason="1/Z bf16 operand"):
                        nc.vector.reciprocal(zbBb[:, 0:256], zbB[:, 0:256])
                        nc.vector.reciprocal(zbBb[:, 256:512], zbB[:, 256:512])
                    emit_norm_post(j, us, (0, 1, 2), zb4, e4)
            # O-projection of the previous chunk and the next chunk's V
            # projection are emitted before this chunk's normalize: the PE
            # queue is in-order, so the norm broadcast matmuls (gated on the
            # batched reciprocal) must sit BEHIND the filler matmuls
            if j == NQ - 1:
                emit_oproj(j - 1, on_act=True)
                # head-3 post, column-split to chase the reciprocal halves
                for half in range(2):
                    csl = slice(half * 256, (half + 1) * 256)
                    rbps = psA.tile([64, 256], F32, tag="proj", name="rbps")
                    nc.tensor.matmul(
                        rbps[:], e2[:, 0:64], zbBb[:, csl], start=True, stop=True
                    )
                    nt3 = wk.tile([64, 256], BF16, tag="nt3", name="nt3", bufs=2)
                    nc.vector.tensor_mul(nt3[:], us[3][0:64, csl], rbps[:])
                    nc.sync.dma_start(
                        ON[
                            64:128,
                            1,
                            j * 512 + half * 256 : j * 512 + (half + 1) * 256,
                        ],
                        nt3[:],
                    )
            else:
                vp = (2 * j + 2, 2 * j + 3)
                # first v-chunk right after the last PV: it depends on
                # nothing recent, bridging the PE over the boundary DVE drain
                emit_vproj(vp[:1])
                # oproj next so its PSUM-freeing ot copies aren't queued on
                # DVE behind this chunk's 3.3us reciprocal; the reciprocal
                # still runs while the PE chews kproj/vproj, so post (at the
                # end of the fillers) never stalls
                if j > 0:
                    emit_oproj(j - 1)
                emit_norm_pre(us, (0, 1, 2, 3), zb4, 32)
                emit_kproj(j + 1)
                emit_vproj(vp[1:])
                emit_norm_post(j, us, (0, 1, 2, 3), zb4, e4)
        emit_oproj(NQ - 1, on_act=True)


def build_nc():
    nc = bacc.Bacc("TRN2", target_bir_lowering=False, debug=False, num_devices=8)
    io = {}
    for name, shape, dt in (
        ("qt", (D, S), BF16),
        ("kt", (D, S), BF16),
        ("vt", (D, S), BF16),
        ("wqt", (128, 2048), BF16),
        ("wkt", (128, 2048), BF16),
        ("wvt", (128, 2048), BF16),
        ("wot", (128, 2048), BF16),
        ("bqc", (128, 2), F32),
        ("bkc", (128, 2), F32),
        ("bvr", (1, DL), BF16),
    ):
        io[name] = nc.dram_tensor(name, shape, dt, kind="ExternalInput")
    io["outp"] = nc.dram_tensor("outp", (D, S), BF16, kind="ExternalOutput")
    with tile.TileContext(nc) as tc:
        _emit(tc, io)
    nc.compile()
    return nc


_NC = None


def _get_nc():
    global _NC
    if _NC is None:
        _NC = build_nc()
    return _NC


def make_in_maps(q, k, v, Wq, bq, Wk, bk, Wv, bv, Wo):
    def cb(x):  # contiguous bf16
        return np.ascontiguousarray(x).astype(NPBF16)

    cf = np.ascontiguousarray
    in_maps = []
    for core in range(8):
        b, g = divmod(core, 4)
        sl = slice(DL * g, DL * (g + 1))
        in_maps.append(
            {
                "qt": cb(q[b].T),
                "kt": cb(k[b].T),
                "vt": cb(v[b].T),
                "wqt": cb(Wq[sl, :].T.reshape(8, 128, DL).transpose(1, 0, 2).reshape(128, 2048)),
                "wkt": cb(Wk[sl, :].T.reshape(8, 128, DL).transpose(1, 0, 2).reshape(128, 2048)),
                "wvt": cb(Wv[sl, :].T.reshape(8, 128, DL).transpose(1, 0, 2).reshape(128, 2048)),
                "wot": cb(Wo[:, sl].T.reshape(2, 128, D).transpose(1, 0, 2).reshape(128, 2048)),
                "bqc": cf(bq[sl].reshape(2, 128).T),
                "bkc": cf(bk[sl].reshape(2, 128).T),
                "bvr": cb(bv[sl].reshape(1, DL)),
            }
        )
    return in_maps


def gather_output(results, bo):
    out = np.empty((B, S, D), np.float32)
    for b in range(B):
        acc = results[4 * b]["outp"].astype(np.float32)
        for g in range(1, 4):
            acc = acc + results[4 * b + g]["outp"].astype(np.float32)
        out[b] = acc.T + bo
    return out


def _np_fallback(q, k, v, mask, Wq, bq, Wk, bk, Wv, bv, Wo, bo):
    # generic-mask reference path; only used if the mask is not causal
    out = np.empty((B, S, D), np.float32)
    m = np.broadcast_to(mask, (B, 1, S, S))
    for b in range(B):
        Q = (q[b] @ Wq.T + bq).reshape(S, H, DK).transpose(1, 0, 2)
        K = (k[b] @ Wk.T + bk).reshape(S, H, DK).transpose(1, 0, 2)
        V = (v[b] @ Wv.T + bv).reshape(S, H, DK).transpose(1, 0, 2)
        o = np.empty((H, S, DK), np.float32)
        for hh in range(H):
            s = (Q[hh] @ K[hh].T) * (DK**-0.5)
            s = np.where(m[b, 0] == 0, -np.inf, s)
            s = s - s.max(axis=-1, keepdims=True)
            e = np.exp(s)
            o[hh] = (e / e.sum(axis=-1, keepdims=True)) @ V[hh]
        out[b] = o.transpose(1, 0, 2).reshape(S, D) @ Wo.T + bo
    return out


def kernel(q, k, v, mask, Wq, bq, Wk, bk, Wv, bv, Wo, bo):
    f32 = np.float32
    q, k, v = (np.asarray(x, f32) for x in (q, k, v))
    Wq, bq, Wk, bk = (np.asarray(x, f32) for x in (Wq, bq, Wk, bk))
    Wv, bv, Wo, bo = (np.asarray(x, f32) for x in (Wv, bv, Wo, bo))
    mask = np.asarray(mask)

    if not np.array_equal(
        np.broadcast_to(mask, (1, 1, S, S))[0, 0] != 0,
        np.tril(np.ones((S, S), bool)),
    ):
        return _np_fallback(q, k, v, mask, Wq, bq, Wk, bk, Wv, bv, Wo, bo)

    nc = _get_nc()
    in_maps = make_in_maps(q, k, v, Wq, bq, Wk, bk, Wv, bv, Wo)
    res = run_bass_kernel_spmd(nc, in_maps, list(range(8)))
    return gather_output(res.results, bo)



# revision 37
# speedup vs baseline: 1.0321x; 1.0321x over previous
"""Multi-head attention (B=2, S=2048, D=1024, H=16, causal) on 8 TRN2 NeuronCores.

Sharding: core c handles batch c//4 and heads [4*(c%4), 4*(c%4)+4) —
data-parallel over batch x tensor-parallel over heads, Megatron-style:
QKV projection weights are column-split (each core computes only its own
heads' features), the output projection is row-split (each core emits a
full-width partial that the host sums).

Per-core device kernel (bf16 matmul operands, fp32 accumulation):
  - Q,K projected feature-major (QT/KT = W_local @ x^T, shape (256, 2048))
    so the scores matmul needs no on-device transposes.
  - V projected in natural (seq, feat) layout with a fused ones-column so
    a single PV matmul produces both attn@V and the softmax denominator.
  - scores^T per (head, q-chunk, key-chunk): K^T-chunk stationary, Q moving.
  - softmax without max-subtraction (scores ~ N(0,1); exp is accurate
    enough), causal handled by skipping upper-triangle key chunks and
    affine_select-masking the 4 diagonal chunk patterns.
  - normalization: the PV accumulator is copied to SBUF immediately (frees
    the PSUM slot so the PE never stalls), all 1/Z reciprocals of a chunk
    run on VectorE at the chunk boundary (DVE reciprocal — ScalarE Ln/Exp
    thrash ACT table sets; custom-DVE ops and partition_broadcast
    mis-execute on HW via this path), and Z is broadcast across 64
    partitions with a K=64 one-hot fp32 matmul (K=1 matmuls read as idle
    to the PE activity monitor and re-throttled the clock).
  - O projection contracts the 256 local features against Wo rows; the
    partial output is written feature-major (1024, 2048) fp32 and the
    host transposes/sums partials and adds bo.

Scheduling notes (measured on HW via neuron-profile): input rows stream as
full 512KB DMAs (4KB/partition descriptors), weights ship pre-arranged for
contiguous DMA, dependency-less warm-up matmuls run during the initial DMAs
so the HAM clock gate is at 8/8 when real work starts, and the K/V/O
projections are interleaved chunk-wise with the attention chunks (causality
only needs K columns and V chunks progressively) as PE-dense filler where
the exp(ACT)-paced attention pipeline would otherwise idle the PE.
"""

import numpy as np
import ml_dtypes

import concourse.bacc as bacc
import concourse.mybir as mybir
import concourse.tile as tile
from concourse.bass_utils import run_bass_kernel_spmd

B, S, D, H = 2, 2048, 1024, 16
DK = D // H           # 64, head dim
DL = 256              # local (per-core) projected features = 4 heads
NHL = 4               # heads per core
NQ = 4                # q-chunks of 512
F32 = mybir.dt.float32
BF16 = mybir.dt.bfloat16
NPBF16 = ml_dtypes.bfloat16


def _emit(tc, io):
    nc = tc.nc
    qt, kt, vt = io["qt"], io["kt"], io["vt"]          # (1024, 2048) bf16
    wqt, wkt, wvt = io["wqt"], io["wkt"], io["wvt"]    # (1024, 256) bf16
    wot = io["wot"]                                    # (256, 1024) bf16
    bqc, bkc = io["bqc"], io["bkc"]                    # (128, 2) f32
    bvr = io["bvr"]                                    # (1, 256) bf16
    outp = io["outp"]                                  # (1024, 2048) bf16 partials
    EXP = mybir.ActivationFunctionType.Exp

    with (
        tc.tile_pool(name="const", bufs=1) as cw,
        tc.tile_pool(name="io", bufs=16) as iop,
        tc.tile_pool(name="big", bufs=1) as big,
        tc.tile_pool(name="work", bufs=3) as wk,
        tc.tile_pool(name="psA", bufs=2, space="PSUM") as psA,
        tc.tile_pool(name="psB", bufs=2, space="PSUM") as psB,
        tc.tile_pool(name="psC", bufs=2, space="PSUM") as psC,
    ):
        ones_sb = cw.tile([128, 128], BF16)
        nc.vector.memset(ones_sb[:], 1.0)
        bq_sb = cw.tile([128, 2], F32)
        nc.sync.dma_start(bq_sb[:], bqc[:, :])
        bk_sb = cw.tile([128, 2], F32)
        nc.sync.dma_start(bk_sb[:], bkc[:, :])
        bv_sb = cw.tile([1, 256], BF16)
        nc.sync.dma_start(bv_sb[:], bvr[:, :])

        # only the Q weights up front — the other weight DMAs are emitted
        # right before their phase so the first projection matmuls start ASAP
        wq_sb = cw.tile([128, 8, 256], BF16)
        nc.sync.dma_start(wq_sb[:], wqt[:, :].rearrange("p (k m) -> p k m", m=256))

        # free PE warm-up: dependency-less matmuls run while the first
        # weight/row DMAs are in flight, so the HAM clock gate is already at
        # 8/8 when the real work begins
        warm = cw.tile([128, 512], BF16, name="warm")
        nc.vector.memset(warm[:], 0.0)
        for _ in range(16):
            wps = psC.tile([128, 512], F32, tag="pv", name="wps")
            nc.tensor.matmul(wps[:], ones_sb[:, :], warm[:], start=True, stop=True)

        QT = big.tile([128, 2, S], BF16)   # [feat%128, feat//128, seq]
        # K^T kept as two half-zeroed copies so the scores matmul contracts
        # over the full 128 partitions (zeros kill the other head's Q rows);
        # K=64 matmuls read as "half-idle" to the PE activity monitor and the
        # clock gate kept re-throttling the whole attention phase.
        KTe = big.tile([128, 2, S], BF16)
        KTo = big.tile([128, 2, S], BF16)
        nc.gpsimd.memset(KTe[64:128, :, :], 0.0)
        nc.gpsimd.memset(KTo[0:64, :, :], 0.0)
        # [key%128, head, key//128, dk | ones | zero-pad]: padded to 128 cols
        # so PV matmuls drive the full array (M=65 looked half-idle to the PE
        # activity monitor); col 64 is the softmax-denominator ones column
        VA = big.tile([128, NHL, 16, 128], BF16)
        ON = big.tile([128, 2, S], BF16)   # normalized attn out, feature-major
        nc.gpsimd.memset(VA[:, :, :, 64:128], 0.0)
        nc.gpsimd.memset(VA[:, :, :, 64:65], 1.0)

        # Z-reciprocal batching: heads' Z rows are gathered onto distinct
        # partitions of one tile so a single DVE RECIPROCAL (3.3us, free-size
        # bound) serves 4 heads instead of one 3.3us op per head. Gathered
        # partitions sit inside an all-ones tile so the K=64 broadcast matmul
        # contracts 1.0*0.0 (not inf*0 = NaN) on the unused partitions.
        # (single-partition writes must sit at 32-aligned partition offsets,
        # so the gathered Z rows live at partitions 32i)
        zb4 = cw.tile([128, 512], F32)  # batched Z rows at partition 32i
        nc.gpsimd.memset(zb4[:], 1.0)
        zbB = cw.tile([64, 512], F32)   # j=3 solo head 3: Z at partition 0
        nc.gpsimd.memset(zbB[:], 1.0)
        # one-hots + 1/Z operands for the broadcast matmul are BF16: an fp32
        # matmul lowers to TWO half-speed LOW/HIGH passes (~2.1us apiece on
        # the PE) vs 213ns for the bf16 one; 1/Z at bf16 costs ~0.2% rms
        e4 = cw.tile([128, 256], BF16)  # one-hot blocks: col block h row 32h
        nc.gpsimd.memset(e4[:], 0.0)
        for h in range(NHL):
            nc.gpsimd.memset(e4[32 * h : 32 * h + 1, 64 * h : 64 * h + 64], 1.0)
        e2 = cw.tile([64, 128], BF16)   # one-hot blocks: col block i row 32i
        nc.gpsimd.memset(e2[:], 0.0)
        for i in range(2):
            nc.gpsimd.memset(e2[32 * i : 32 * i + 1, 64 * i : 64 * i + 64], 1.0)
        zb4b = cw.tile([128, 512], BF16)  # bf16 casts of the 1/Z tiles
        zbBb = cw.tile([64, 512], BF16)
        zcast = {id(zb4): zb4b, id(zbB): zbBb}

        # ---- Q/K projections, feature-major: dst[:, m, n] = W_local @ x^T ----
        # inputs stream as full 512KB rows (4KB/partition descriptors — small
        # per-partition runs were tanking HW-DGE efficiency)
        wk_sb = cw.tile([128, 8, 256], BF16)
        wv_sb = cw.tile([128, 8, 256], BF16)
        wo_sb = cw.tile([128, 2, 1024], BF16)
        # inputs live in one [128, 8, S] tile each so a single multi-dim DMA
        # moves a whole column chunk of all 8 k-rows (1 issue instead of 8 —
        # SP issue cost is ~0.6us per descriptor). q streams in quarters on
        # the (early-idle) Scalar DMA queue: Qproj chunk n is gated only on
        # quarter n, so the PE gets real work ~4.5us in instead of ~15us.
        # fused [128,8,512] quarter-DMAs: slow to issue (~4us of descriptor
        # generation) but the large burst is measurably better for HW-DGE
        # throughput than 8 per-row slices; Qproj chunk n gates on quarter n
        qrs = iop.tile([128, 8, S], BF16, tag="xin", name="qrs", bufs=3)
        qsrc = qt[:, :].rearrange("(k p) c -> p k c", p=128)
        for n in range(NQ):
            nsl = slice(n * 512, (n + 1) * 512)
            nc.scalar.dma_start(qrs[:, :, nsl], qsrc[:, :, nsl])
        for n in range(NQ):
            pm = [
                psA.tile([128, 512], F32, tag="proj", name=f"pm{m}")
                for m in range(2)
            ]
            for k in range(8):
                for m in range(2):
                    nc.tensor.matmul(
                        pm[m][:],
                        wq_sb[:, k, m * 128 : (m + 1) * 128],
                        qrs[:, k, n * 512 : (n + 1) * 512],
                        start=(k == 0),
                        stop=(k == 7),
                    )
            for m in range(2):
                nc.vector.tensor_scalar_add(
                    QT[:, m, n * 512 : (n + 1) * 512], pm[m][:], bq_sb[:, m : m + 1]
                )

        # K projection split per q-chunk: attention chunk j only needs K
        # columns up to (j+1)*512, so chunks n>=1 are emitted between the
        # attention chunks below (PE-dense filler for the exp-paced phase)
        nc.sync.dma_start(wk_sb[:], wkt[:, :].rearrange("p (k m) -> p k m", m=256))
        krs = iop.tile([128, 8, S], BF16, tag="xin", name="krs", bufs=3)
        ksrc = kt[:, :].rearrange("(k p) c -> p k c", p=128)
        for n in range(NQ):
            nsl = slice(n * 512, (n + 1) * 512)
            nc.scalar.dma_start(krs[:, :, nsl], ksrc[:, :, nsl])

        def emit_kproj(n):
            pm = [
                psA.tile([128, 512], F32, tag="proj", name=f"km{m}")
                for m in range(2)
            ]
            for k in range(8):
                for m in range(2):
                    nc.tensor.matmul(
                        pm[m][:],
                        wk_sb[:, k, m * 128 : (m + 1) * 128],
                        krs[:, k, n * 512 : (n + 1) * 512],
                        start=(k == 0),
                        stop=(k == 7),
                    )
            for m in range(2):
                sl = slice(n * 512, (n + 1) * 512)
                nc.vector.tensor_scalar_add(
                    KTe[0:64, m, sl], pm[m][0:64, :], bk_sb[0:64, m : m + 1]
                )
                nc.vector.tensor_scalar_add(
                    KTo[64:128, m, sl], pm[m][64:128, :], bk_sb[64:128, m : m + 1]
                )

        emit_kproj(0)

        # ---- V projection, natural layout, bias via K=1 ones matmul ----
        # emitted in sp-pairs interleaved with the attention chunks below:
        # attention is exp(ACT)-paced, so V matmuls fill the PE micro-idles
        # that would otherwise re-throttle the clock gate
        nc.sync.dma_start(wv_sb[:], wvt[:, :].rearrange("p (k m) -> p k m", m=256))
        # vrs shares the xin ring with qrs: the v DMAs wait out Qproj's last
        # read (~15us) and still land well before vproj needs them (~25us).
        # Quarters: vproj sp-pair (2n, 2n+1) needs only v columns quarter n.
        # own buffer (bufs=3 ring): v must not wait out Qproj's reads of q
        vrs = iop.tile([128, 8, S], BF16, tag="xin", name="vrs", bufs=3)
        vsrc = vt[:, :].rearrange("(k p) c -> p k c", p=128)
        for n in range(NQ):
            nsl = slice(n * 512, (n + 1) * 512)
            nc.sync.dma_start(vrs[:, :, nsl], vsrc[:, :, nsl])

        def emit_vproj(sps):
            for sp in sps:
                pvps = psA.tile([128, 512], F32, tag="proj", name="pvps")
                for k in range(8):
                    for half in range(2):
                        s = sp * 256 + half * 128
                        nc.tensor.matmul(
                            pvps[:, half * 256 : (half + 1) * 256],
                            vrs[:, k, s : s + 128],
                            wv_sb[:, k, :],
                            start=(k == 0 and half == 0),
                            stop=False,
                        )
                for half in range(2):
                    nc.tensor.matmul(
                        pvps[:, half * 256 : (half + 1) * 256],
                        ones_sb[0:1, 0:128],
                        bv_sb[:],
                        start=False,
                        stop=(half == 1),
                    )
                for half in range(2):
                    s = sp * 2 + half
                    nc.vector.tensor_copy(
                        VA[:, :, s, 0:64],
                        pvps[:, half * 256 : (half + 1) * 256].rearrange(
                            "p (h d) -> p h d", d=64
                        ),
                    )

        emit_vproj((0, 1))

        # wo is not needed until oproj(0) (~55us): defer its DMA out of
        # the congested first-25us HBM window
        nc.sync.dma_start(wo_sb[:], wot[:, :].rearrange("p (c m) -> p c m", m=1024))

        # ---- attention + output projection, q-chunk-major for overlap ----
        # O-proj for chunk j is emitted after attention chunk j+1 so the PE
        # never waits on the normalize chain (reciprocal on DVE) of chunk j.
        def emit_oproj(j, ms=range(8), on_act=False):
            for m in ms:
                po = psA.tile([128, 512], F32, tag="proj", name="po")
                for c in range(2):
                    nc.tensor.matmul(
                        po[:],
                        wo_sb[:, c, m * 128 : (m + 1) * 128],
                        ON[:, c, j * 512 : (j + 1) * 512],
                        start=(c == 0),
                        stop=(c == 1),
                    )
                # bf16 partials: host upcasts before the 4-way sum; halves
                # the 8MB/core output DMA and the end-of-kernel drain
                ot = wk.tile([128, 512], BF16, tag="ot", name="ot")
                if on_act:
                    # the last two O-projections run after the exps are done:
                    # their PSUM-freeing copies go on the then-idle ScalarE
                    # (COPY is in every ACT table set - no thrash) so the PE
                    # never paces on the DVE queue at the tail
                    nc.scalar.activation(
                        ot[:], po[:], mybir.ActivationFunctionType.Copy
                    )
                    nc.gpsimd.dma_start(
                        outp[m * 128 : (m + 1) * 128, j * 512 : (j + 1) * 512],
                        ot[:],
                    )
                else:
                    nc.vector.tensor_copy(ot[:], po[:])
                    nc.sync.dma_start(
                        outp[m * 128 : (m + 1) * 128, j * 512 : (j + 1) * 512],
                        ot[:],
                    )

        # normalize a batch of heads, split in two so the PE-queue placement
        # of the broadcast matmuls can be decoupled from the reciprocal:
        #   pre:  gather Z rows onto distinct partitions of zb (tiny
        #         SBUF->SBUF DMAs — engines can't cross partitions) + one
        #         batched DVE reciprocal (3.3us, free-size bound) for the
        #         whole batch instead of one 3.3us reciprocal per head
        #   post: K=64/128 one-hot matmul broadcast + DVE scale into ON
        def emit_norm_pre(us, heads, zb, stride):
            for i, h in enumerate(heads):
                nc.sync.dma_start(
                    zb[stride * i : stride * i + 1, :], us[h][64:65, :]
                )
            # reciprocal converts to bf16 on the way out (DVE auto-converts)
            # so the broadcast matmul gets its full-speed operand directly
            with nc.allow_low_precision(reason="1/Z bf16 broadcast operand"):
                nc.vector.reciprocal(zcast[id(zb)][:], zb[:])

        def emit_norm_post(j, us, heads, zb, eh):
            for i, h in enumerate(heads):
                rbps = psA.tile([64, 512], F32, tag="proj", name="rbps")
                nc.tensor.matmul(
                    rbps[:],
                    eh[:, 64 * i : 64 * i + 64],
                    zcast[id(zb)][:],
                    start=True,
                    stop=True,
                )
                if h % 2 == 0:
                    nc.vector.tensor_mul(
                        ON[0:64, h // 2, j * 512 : (j + 1) * 512], us[h][0:64, :], rbps[:]
                    )
                else:
                    nt = wk.tile([64, 512], BF16, tag="nt", name="nt")
                    nc.vector.tensor_mul(nt[:], us[h][0:64, :], rbps[:])
                    nc.sync.dma_start(
                        ON[64:128, h // 2, j * 512 : (j + 1) * 512], nt[:]
                    )

        for j in range(NQ):
            nkc = 4 * (j + 1)
            us = []
            for h in range(NHL):
                ch = h // 2
                pvp = psC.tile([128, 512], F32, tag="pv", name="pvp")
                # exp batched over 2 key-chunks (one 2-bank PSUM tile) to
                # amortize the ~350ns fixed cost per ACTIVATE. Diagonal
                # key-chunks (kc >= 4j) skip their leading fully-masked
                # columns: q columns < 128*(kc-4j) attend only to earlier
                # keys, so scores/exp/select/PV all trim to [c0:512].
                for kp in range(nkc // 2):
                    scp = psB.tile([128, 1024], F32, tag="sc", name="scp")
                    ktz = KTe if h % 2 == 0 else KTo
                    c0s = []
                    for half in range(2):
                        kc = 2 * kp + half
                        c0 = max(0, 128 * (kc - 4 * j))
                        c0s.append(c0)
                        nc.tensor.matmul(
                            scp[:, half * 512 + c0 : (half + 1) * 512],
                            ktz[:, ch, kc * 128 : (kc + 1) * 128],
                            QT[:, ch, j * 512 + c0 : (j + 1) * 512],
                            start=True,
                            stop=True,
                        )
                    et = wk.tile([128, 1024], BF16, tag="exp", name="et", bufs=4)
                    if c0s[1] == 0:
                        nc.scalar.activation(
                            et[:], scp[:], EXP, scale=float(DK) ** -0.5
                        )
                    else:
                        for half in range(2):
                            sl = slice(half * 512 + c0s[half], (half + 1) * 512)
                            nc.scalar.activation(
                                et[:, sl], scp[:, sl], EXP, scale=float(DK) ** -0.5
                            )
                    for half in range(2):
                        kc = 2 * kp + half
                        if kc >= 4 * j:
                            c0 = c0s[half]
                            nc.gpsimd.affine_select(
                                out=et[:, half * 512 + c0 : (half + 1) * 512],
                                in_=et[:, half * 512 + c0 : (half + 1) * 512],
                                compare_op=mybir.AluOpType.is_ge,
                                fill=0.0,
                                base=0,
                                pattern=[[1, 512 - c0]],
                                channel_multiplier=-1,
                            )
                    for half in range(2):
                        kc = 2 * kp + half
                        c0 = c0s[half]
                        nc.tensor.matmul(
                            pvp[:, c0:512],
                            VA[:, h, kc, :],
                            et[:, half * 512 + c0 : (half + 1) * 512],
                            start=(kc == 0),
                            stop=(kc == nkc - 1),
                        )
                # copy the PV accumulator to SBUF immediately — freeing the
                # pv PSUM slot fast keeps the PE from stalling (and HAM from
                # re-throttling the clock); normalization is deferred below.
                u = wk.tile([128, 512], F32, tag="u", name="u", bufs=6)
                nc.vector.tensor_copy(u[:], pvp[:])
                us.append(u)
                # last chunk: normalize in 2-head batches, pre (reciprocal)
                # right after the pair's PVs land and post (PE broadcast)
                # only after another head's attention has filled the PE
                # queue, so the O-projections aren't serialized behind one
                # end-of-chunk reciprocal
                if j == NQ - 1 and h == 2:
                    # heads 0-2 batch: reciprocal runs during h3's attention
                    emit_norm_pre(us, (0, 1, 2), zb4, 32)
                if j == NQ - 1 and h == 3:
                    # solo head 3: column-split reciprocal so the first half
                    # of 1/Z is ready ~1.7us sooner on the tail chain
                    nc.sync.dma_start(zbB[0:1, :], us[3][64:65, :])
                    with nc.allow_low_precision(reason="1/Z bf16 operand"):
                        nc.vector.reciprocal(zbBb[:, 0:256], zbB[:, 0:256])
                        nc.vector.reciprocal(zbBb[:, 256:512], zbB[:, 256:512])
                    emit_norm_post(j, us, (0, 1, 2), zb4, e4)
            # O-projection of the previous chunk and the next chunk's V
            # projection are emitted before this chunk's normalize: the PE
            # queue is in-order, so the norm broadcast matmuls (gated on the
            # batched reciprocal) must sit BEHIND the filler matmuls
            if j == NQ - 1:
                emit_oproj(j - 1, on_act=True)
                # head-3 post, column-split to chase the reciprocal halves
                for half in range(2):
                    csl = slice(half * 256, (half + 1) * 256)
                    rbps = psA.tile([64, 256], F32, tag="proj", name="rbps")
                    nc.tensor.matmul(
                        rbps[:], e2[:, 0:64], zbBb[:, csl], start=True, stop=True
                    )
                    nt3 = wk.tile([64, 256], BF16, tag="nt3", name="nt3", bufs=2)
                    nc.vector.tensor_mul(nt3[:], us[3][0:64, csl], rbps[:])
                    nc.sync.dma_start(
                        ON[
                            64:128,
                            1,
                            j * 512 + half * 256 : j * 512 + (half + 1) * 256,
                        ],
                        nt3[:],
                    )
            else:
                vp = (2 * j + 2, 2 * j + 3)
                # first v-chunk right after the last PV: it depends on
                # nothing recent, bridging the PE over the boundary DVE drain
                emit_vproj(vp[:1])
                # oproj next so its PSUM-freeing ot copies aren't queued on
                # DVE behind this chunk's 3.3us reciprocal; the reciprocal
                # still runs while the PE chews kproj/vproj, so post (at the
                # end of the fillers) never stalls
                if j > 0:
                    emit_oproj(j - 1)
                emit_norm_pre(us, (0, 1, 2, 3), zb4, 32)
                emit_kproj(j + 1)
                emit_vproj(vp[1:])
                emit_norm_post(j, us, (0, 1, 2, 3), zb4, e4)
        emit_oproj(NQ - 1, on_act=True)


def build_nc():
    nc = bacc.Bacc("TRN2", target_bir_lowering=False, debug=False, num_devices=8)
    io = {}
    for name, shape, dt in (
        ("qt", (D, S), BF16),
        ("kt", (D, S), BF16),
        ("vt", (D, S), BF16),
        ("wqt", (128, 2048), BF16),
        ("wkt", (128, 2048), BF16),
        ("wvt", (128, 2048), BF16),
        ("wot", (128, 2048), BF16),
        ("bqc", (128, 2), F32),
        ("bkc", (128, 2), F32),
        ("bvr", (1, DL), BF16),
    ):
        io[name] = nc.dram_tensor(name, shape, dt, kind="ExternalInput")
    io["outp"] = nc.dram_tensor("outp", (D, S), BF16, kind="ExternalOutput")
    with tile.TileContext(nc) as tc:
        _emit(tc, io)
    nc.compile()
    return nc


_NC = None


def _get_nc():
    global _NC
    if _NC is None:
        _NC = build_nc()
    return _NC


def make_in_maps(q, k, v, Wq, bq, Wk, bk, Wv, bv, Wo):
    def cb(x):  # contiguous bf16
        return np.ascontiguousarray(x).astype(NPBF16)

    cf = np.ascontiguousarray
    in_maps = []
    for core in range(8):
        b, g = divmod(core, 4)
        sl = slice(DL * g, DL * (g + 1))
        in_maps.append(
            {
                "qt": cb(q[b].T),
                "kt": cb(k[b].T),
                "vt": cb(v[b].T),
                "wqt": cb(Wq[sl, :].T.reshape(8, 128, DL).transpose(1, 0, 2).reshape(128, 2048)),
                "wkt": cb(Wk[sl, :].T.reshape(8, 128, DL).transpose(1, 0, 2).reshape(128, 2048)),
                "wvt": cb(Wv[sl, :].T.reshape(8, 128, DL).transpose(1, 0, 2).reshape(128, 2048)),
                "wot": cb(Wo[:, sl].T.reshape(2, 128, D).transpose(1, 0, 2).reshape(128, 2048)),
                "bqc": cf(bq[sl].reshape(2, 128).T),
                "bkc": cf(bk[sl].reshape(2, 128).T),
                "bvr": cb(bv[sl].reshape(1, DL)),
            }
        )
    return in_maps


def gather_output(results, bo):
    out = np.empty((B, S, D), np.float32)
    for b in range(B):
        acc = results[4 * b]["outp"].astype(np.float32)
        for g in range(1, 4):
            acc = acc + results[4 * b + g]["outp"].astype(np.float32)
        out[b] = acc.T + bo
    return out


def _np_fallback(q, k, v, mask, Wq, bq, Wk, bk, Wv, bv, Wo, bo):
    # generic-mask reference path; only used if the mask is not causal
    out = np.empty((B, S, D), np.float32)
    m = np.broadcast_to(mask, (B, 1, S, S))
    for b in range(B):
        Q = (q[b] @ Wq.T + bq).reshape(S, H, DK).transpose(1, 0, 2)
        K = (k[b] @ Wk.T + bk).reshape(S, H, DK).transpose(1, 0, 2)
        V = (v[b] @ Wv.T + bv).reshape(S, H, DK).transpose(1, 0, 2)
        o = np.empty((H, S, DK), np.float32)
        for hh in range(H):
            s = (Q[hh] @ K[hh].T) * (DK**-0.5)
            s = np.where(m[b, 0] == 0, -np.inf, s)
            s = s - s.max(axis=-1, keepdims=True)
            e = np.exp(s)
            o[hh] = (e / e.sum(axis=-1, keepdims=True)) @ V[hh]
        out[b] = o.transpose(1, 0, 2).reshape(S, D) @ Wo.T + bo
    return out


def kernel(q, k, v, mask, Wq, bq, Wk, bk, Wv, bv, Wo, bo):
    f32 = np.float32
    q, k, v = (np.asarray(x, f32) for x in (q, k, v))
    Wq, bq, Wk, bk = (np.asarray(x, f32) for x in (Wq, bq, Wk, bk))
    Wv, bv, Wo, bo = (np.asarray(x, f32) for x in (Wv, bv, Wo, bo))
    mask = np.asarray(mask)

    if not np.array_equal(
        np.broadcast_to(mask, (1, 1, S, S))[0, 0] != 0,
        np.tril(np.ones((S, S), bool)),
    ):
        return _np_fallback(q, k, v, mask, Wq, bq, Wk, bk, Wv, bv, Wo, bo)

    nc = _get_nc()
    in_maps = make_in_maps(q, k, v, Wq, bq, Wk, bk, Wv, bv, Wo)
    res = run_bass_kernel_spmd(nc, in_maps, list(range(8)))
    return gather_output(res.results, bo)

